# revision 49
# baseline (speedup 1.0000x reference)
"""FBGCN layer kernel for 8 Trainium2 NeuronCores.

out = aL * GCNConv(x, edge_index; W_conv, b_conv) + aH * (Lsym @ relu(x @ W_high.T))

Sharding: 1D row-partition of output nodes across 8 cores (1536 rows each).
Per core (no cross-core communication):
  - A0: Y = relu(x @ W_high.T)/256 (fp16) and xw = (x @ W_conv.T)/8 (fp8e4)
    for ALL nodes; xw lands in a DRAM scratch with 256B-strided rows.
  - Hh: the core's column slice of (256*aH*Lsym).T streams in as float8e3
    (E3M4) into a single SBUF-resident tile (147KB/partition) via chunked
    DMAs that never wait on compute; Lsym chunks are the STATIONARY matmul
    operand and Y the moving one, so PSUM accumulates out[128, 12*64]
    directly in the output orientation (no final transpose).  Optionally
    the first KB16 k-blocks stream in fp16 for extra precision margin.
  - Hl (GCN): per-edge source rows of xw are fetched with 64B-payload
    dma_gathers (stride 256B, SWDGE semaphore lanes - independent of the
    HWDGE lanes the ls stream uses), then multiplied by a host-built fp8e4
    block-diagonal "segment matrix" (8*aL*norm weights, one resident tile)
    on the PE per 128-target block; self loops ride along as edges; gather
    padding slots point at row 0 and carry zero weights.
  - Final: two DVE adds combine PSUM Hh + Hl(+bias) and two DMAs store
    out[128, 12*64] f32 (host de-interleaves to [1536, 64]).
"""

import numpy as np
import ml_dtypes

import concourse.bacc as bacc
import concourse.mybir as mybir
import concourse.tile as tile
from concourse.bass_utils import run_bass_kernel_spmd

N, E, D = 12288, 196608, 64
NCORES = 8
M = N // NCORES          # 1536 output rows per core
MB = M // 128            # 12 target blocks per core
KB = N // 128            # 96 contraction blocks
SCR_W = 256              # scratch row stride in bytes (gather requirement)
G = 32                   # target-group width
GPB = 128 // G
LS_SCALE = 256.0         # Lsym pre-scale (folded back via Y)
XW_SCALE = 8.0           # xw stored /8, seg weights *8
KB16 = 0                 # leading k-blocks streamed in fp16 (accuracy knob)
P8 = 8                   # fp8 lsym k-blocks per DMA chunk
P16 = 4                  # fp16 lsym k-blocks per DMA tile

F32 = mybir.dt.float32
F16 = mybir.dt.float16
FP8L = mybir.dt.float8e3     # Lsym stream (E3M4)
FP8M = mybir.dt.float8e4     # messages + segment weights (E4M3)
NP8L = ml_dtypes.float8_e3m4
NP8M = ml_dtypes.float8_e4m3
AFT = mybir.ActivationFunctionType


def _scratch_row(n):
    """Node n -> scratch row (partition-contiguous layout)."""
    return (n % 128) * KB + n // 128


def _raw_dma_gather(gp, out_ap, in_ap, idxs_ap, num_idxs, elem_size,
                    elem_step):
    """bass.dma_gather minus the elem_size_bytes % 256 assert (the payload
    may be any size; only the source row stride must be 256B-aligned).
    Non-transpose, DRAM source, single_packet=False."""
    stride_bytes = elem_step * mybir.dt.size(in_ap.dtype)
    stride_bytes_256, rem = divmod(stride_bytes, 256)
    assert rem == 0 and 0 < stride_bytes_256 < 256
    _in_ap = gp.lower_ap_dma(in_ap, for_custom_bir_dma=True)
    _idxs_ap = gp.lower_ap(idxs_ap)
    _out_ap = gp.lower_ap(out_ap)
    return gp.add_instruction(
        mybir.InstDMAGatherAnt(
            name=gp.bass.get_next_instruction_name(),
            ins=[*_in_ap, _idxs_ap, gp.lower_val_access(gp.to_reg(num_idxs))],
            outs=[_out_ap],
            transpose=False,
            num_idxs=num_idxs,
            elem_size=elem_size,
            stride_bytes_256=stride_bytes_256,
            gen_mode=0,
            single_packet=False,
            queue_num=0,
            sbuf_tokens_per_rank=0,
            sbuf_free_dim_per_rank=0,
            sbuf_free_dim_pad_per_rank=0,
            sbuf_byte_offset=0,
        )
    )


def _build_program(C: int, kb16=KB16, gcn_sched=None, msg_bufs=3):
    """Build the SPMD Bass program. C = edge chunks (of 128 slots) per
    G-target group."""
    QB = GPB * C             # chunks per 128-target block
    S = MB * QB * 128        # edge slots per core
    n8 = KB - kb16
    nc = bacc.Bacc("TRN2", target_bir_lowering=False, debug=False,
                   num_devices=NCORES)

    lsymT8 = nc.dram_tensor("lsymT8", [n8 * 128, M], FP8L,
                            kind="ExternalInput")
    if kb16:
        lsymT16 = nc.dram_tensor("lsymT16", [kb16 * 128, M], F16,
                                 kind="ExternalInput")
    xT = nc.dram_tensor("xT", [D, N], F16, kind="ExternalInput")
    wt2 = nc.dram_tensor("wt2", [D, 2 * D], F16, kind="ExternalInput")
    segP = nc.dram_tensor("segP", [128, MB * QB * G], FP8M,
                          kind="ExternalInput")
    gidx = nc.dram_tensor("gidx", [128, S // 16], mybir.dt.int16,
                          kind="ExternalInput")
    bias128 = nc.dram_tensor("bias128", [128, D], F32, kind="ExternalInput")
    outp = nc.dram_tensor("out", [128, MB * D], F32, kind="ExternalOutput")

    # GCN blocks are processed in pairs (one gather per pair halves the
    # fixed SWDGE prep overhead); pairs are scheduled so each gather lands
    # before the in-order PE queue reaches the pair's matmuls.
    if gcn_sched is None:
        gcn_sched = [36 + 12 * p for p in range(MB // 4)]
    assert len(gcn_sched) == MB // 4 and all(0 <= k < KB for k in gcn_sched)
    sched = {}
    for p, k in enumerate(gcn_sched):
        sched.setdefault(k, []).append(p)

    with tile.TileContext(nc) as tc:
        with (
            tc.tile_pool(name="consts", bufs=1) as consts,
            tc.tile_pool(name="dram", bufs=1, space="DRAM") as dram,
            tc.tile_pool(name="ls16", bufs=2) as ls16_pool,
            tc.tile_pool(name="msg", bufs=msg_bufs) as msg_pool,
            tc.tile_pool(name="psb", bufs=1, space="PSUM") as ps_big,
            tc.tile_pool(name="psa", bufs=2, space="PSUM") as ps_a0,
            tc.tile_pool(name="pss", bufs=2, space="PSUM") as ps_small,
        ):
            # ---- persistent tiles ----
            # The whole fp8 Lsym slice stays resident in SBUF (147KB of the
            # ~208KB partition budget): its chunk DMAs never wait on compute,
            # so the DMA engine is never stalled by buffer recycling.  xT is
            # only needed during A0, so it lives aliased in the tail bytes of
            # ls_all (the framework's WAR tracking serializes the two
            # overlapping ls chunks behind A0's last read).
            ls_all = consts.tile([128, n8 * M], FP8L, tag="lsall")
            lsva = ls_all[:].rearrange("p (t m) -> p t m", t=n8)
            wt2_sb = consts.tile([D, 2 * D], F16, tag="wt2")
            nc.sync.dma_start(wt2_sb[:], wt2[:])
            xt_sb = ls_all[0:D, n8 * M - 2 * N:n8 * M].bitcast(F16)
            XC = N // 4
            for c in range(4):
                nc.sync.dma_start(xt_sb[:, c * XC:(c + 1) * XC],
                                  xT[:, c * XC:(c + 1) * XC])
            bias_sb = consts.tile([128, D], F32, tag="bias")
            nc.scalar.dma_start(bias_sb[:], bias128[:])
            idx_sb = consts.tile([128, S // 16], mybir.dt.int16, tag="idx")
            nc.scalar.dma_start(idx_sb[:], gidx[:])
            seg_all = consts.tile([128, MB * QB * G], FP8M, tag="segall")
            nc.scalar.dma_start(seg_all[:], segP[:])
            y_all = consts.tile([128, KB * D], F16, tag="yall")
            xw_all = consts.tile([128, KB * D], FP8M, tag="xwall")
            hl_sb = consts.tile([128, MB * D], F32, tag="hl")
            ob_sb = consts.tile([128, MB * D], F32, tag="ob")

            scratch = dram.tile([N, SCR_W], FP8M, tag="scr")

            # ---- phase A0: Y and xw for all nodes ----
            # 8 node-blocks share one PSUM bank pair so the activation/copy
            # run 512-wide (amortizes PSUM access latency and semaphore
            # rounds 8x); start=True per 2KB bank (q==0 and q==4) since a
            # start marks its whole zero-region pending.
            # Two interleaved passes of 8-block batches: xw iters feed the
            # DVE (scale copy), y iters feed the ACT (relu) - both chains
            # run concurrently on alternating PSUM bufs while the PE streams
            # all 192 matmuls back-to-back (continuous dispatch keeps the PE
            # p-state ramped; one 512-wide post-op per batch amortizes the
            # PSUM access latency 8x).
            A0B = 8
            for k8 in range(KB // A0B):
                psx = ps_a0.tile([128, A0B * D], F32, tag="a0")
                for q in range(A0B):
                    nc.tensor.matmul(
                        psx[:, q * D:(q + 1) * D],
                        lhsT=xt_sb[:, (A0B * k8 + q) * 128:
                                   (A0B * k8 + q + 1) * 128],
                        rhs=wt2_sb[:, D:2 * D],
                        start=(q == 0), stop=(q == A0B - 1),
                        skip_group_check=True,
                    )
                nc.vector.tensor_scalar_mul(
                    xw_all[:, k8 * A0B * D:(k8 + 1) * A0B * D],
                    psx[:], 1.0 / XW_SCALE)
                psy = ps_a0.tile([128, A0B * D], F32, tag="a0")
                for q in range(A0B):
                    nc.tensor.matmul(
                        psy[:, q * D:(q + 1) * D],
                        lhsT=xt_sb[:, (A0B * k8 + q) * 128:
                                   (A0B * k8 + q + 1) * 128],
                        rhs=wt2_sb[:, 0:D],
                        start=(q == 0), stop=(q == A0B - 1),
                        skip_group_check=True,
                    )
                nc.scalar.activation(
                    y_all[:, k8 * A0B * D:(k8 + 1) * A0B * D],
                    psy[:], AFT.Relu, scale=1.0 / LS_SCALE)


            # ---- main stream + interleaved GCN blocks ----
            # start=False throughout: a start=True matmul marks its whole 2KB
            # PSUM zero-region pending-zero, wiping bank-mates' accumulation.
            hhps = ps_big.tile([128, MB * D], F32, tag="hh")
            nc.vector.memset(hhps[:], 0)

            GQ = 4               # blocks per gather
            msg_tiles = []
            for q in range(MB // GQ):
                msg_q = consts.tile([128, GQ * QB * D], FP8M, tag=f"msg{q}",
                                    name=f"msg{q}")
                msg_tiles.append(msg_q)


            def emit_gcn_quad(p):
                b0 = GQ * p
                msg_sb = msg_tiles[p]
                gather_insts.append(_raw_dma_gather(
                    nc.gpsimd,
                    msg_sb[:].rearrange("p (c f) -> p c f", c=GQ * QB),
                    scratch[:, 0:D],
                    idx_sb[:, b0 * QB * 8:(b0 + GQ) * QB * 8],
                    GQ * QB * 128, D, SCR_W,
                ))
                for i in range(GQ):
                    b = b0 + i
                    segv = seg_all[:, b * QB * G:(b + 1) * QB * G] \
                        .rearrange("p (q t) -> p q t", t=G)
                    msgv = msg_sb[:, i * QB * D:(i + 1) * QB * D] \
                        .rearrange("p (c f) -> p c f", c=QB)
                    for g in range(GPB):
                        hl = ps_small.tile([G, D], F32, tag="ps")
                        for c in range(C):
                            q = g * C + c
                            nc.tensor.matmul(
                                hl[:], lhsT=segv[:, q, :], rhs=msgv[:, q, :],
                                start=(c == 0), stop=(c == C - 1))
                        # bias add folded into the PSUM->SBUF copy
                        nc.vector.tensor_add(
                            hl_sb[G * g:G * (g + 1), b * D:(b + 1) * D],
                            hl[:], bias_sb[G * g:G * (g + 1), :])

            # scratch quarters go on the SP queue interleaved between the
            # first ls chunks: SP head-of-line blocking paces the ls stream
            # so the scratch lands early in the DMA FIFO and the gathers
            # (which need the full scratch) can start by ~20us.
            def emit_scratch_quarter(jq):
                h0 = jq * (KB // 4)
                h1 = h0 + KB // 4
                nc.sync.dma_start(
                    scratch[:, 0:D]
                    .rearrange("(p a) f -> p a f", p=128)[:, h0:h1, :],
                    xw_all[:, h0 * D:h1 * D]
                    .rearrange("p (a f) -> p a f", a=KB // 4),
                )

            # fp8 chunk plan: 8s with a tapered tail so the last DMAs are
            # small (shrinks the post-stream PE wind-down)
            packs = []
            rem = n8
            while rem > 8:
                packs.append(8)
                rem -= 8
            while rem > 1:
                packs.append(rem // 2)
                rem -= rem // 2
            if rem:
                packs.append(rem)
            chunk_start = {}
            s = 0
            for p in packs:
                chunk_start[s] = (len(chunk_start), p)
                s += p

            # Pace mid-stream ls chunk DMA *requests* on gather-prep
            # progress (Pool token increments).  The DMA engine FIFO serves
            # strict request order, so without pacing all resident-tile
            # chunks are requested at t~0 and the mid-stream gathers queue
            # behind the whole 52us ls stream, starving the in-order PE.
            pace = nc.alloc_semaphore("ls_pace")
            pace_waits = []
            gather_insts = []

            ls_sb = None
            lsv = None
            for kb in range(KB):
                if kb < kb16:
                    if kb % P16 == 0:
                        ls_sb = ls16_pool.tile([128, P16 * M], F16, tag="l16")
                        r0 = kb * 128
                        nc.sync.dma_start(
                            ls_sb[:].rearrange("p (t m) -> p t m", t=P16),
                            lsymT16[r0:r0 + P16 * 128, :]
                            .rearrange("(t p) m -> p t m", p=128),
                        )
                        lsv = ls_sb[:].rearrange("p (t m) -> p t m", t=P16)
                    lst = lsv[:, kb % P16, :]
                else:
                    kk = kb - kb16
                    if kk in chunk_start:
                        ci, pk = chunk_start[kk]
                        r0 = kk * 128
                        di = nc.sync.dma_start(
                            lsva[:, kk:kk + pk, :],
                            lsymT8[r0:r0 + pk * 128, :]
                            .rearrange("(t p) m -> p t m", p=128),
                        )
                        if ci in (5, 6, 7):
                            pace_waits.append(di)
                    lst = lsva[:, kk, :]
                if kb == 1:
                    for jq in range(4):
                        emit_scratch_quarter(jq)
                for mc in range(MB):
                    nc.tensor.matmul(
                        hhps[:, mc * D:(mc + 1) * D],
                        lhsT=lst[:, mc * 128:(mc + 1) * 128],
                        rhs=y_all[:, kb * D:(kb + 1) * D],
                        start=False, stop=(kb == KB - 1),
                        skip_group_check=True,
                    )
                if kb in sched:
                    for p in sched[kb]:
                        emit_gcn_quad(p)

            # ---- final combine + store (split so add/store pipeline) ----
            H = MB * D // 2
            for h in range(2):
                nc.vector.tensor_add(ob_sb[:, h * H:(h + 1) * H],
                                     hhps[:, h * H:(h + 1) * H],
                                     hl_sb[:, h * H:(h + 1) * H])
                nc.sync.dma_start(outp[:, h * H:(h + 1) * H],
                                  ob_sb[:, h * H:(h + 1) * H])

    nc.compile()
    # Post-compile: gate the mid-stream ls chunk DMAs on their pacing
    # gather's framework-assigned SWDGE lane semaphore (+16 at gather DMA
    # completion).  Attached directly to the DMA instruction so the tile
    # scheduler cannot hoist the gate.
    assert len(pace_waits) == len(gather_insts) == MB // 4
    import concourse.mybir as _mb
    for wi, gi in zip(pace_waits, gather_insts):
        upd = gi.ins.sync_info.on_update[0]
        si = wi.ins.sync_info
        nw = _mb.SyncWait(sync_type="semaphore", id=upd.id, ant_name=None,
                          wait_mode="sem-ge-imm", wait_value=16,
                          wait_reg=None)
        si.on_wait = list(si.on_wait) + [nw]
    return nc


def _prepare_host(x, edge_index, Lsym, W_high, W_conv, b_conv, aL, aH):
    """Shard + preprocess inputs. Returns (in_maps, C)."""
    x = np.asarray(x, np.float32)
    edge_index = np.asarray(edge_index)
    Lsym = np.asarray(Lsym, np.float32)
    W_high = np.asarray(W_high, np.float32)
    W_conv = np.asarray(W_conv, np.float32)
    b_conv = np.asarray(b_conv, np.float32)
    aL = float(np.asarray(aL))
    aH = float(np.asarray(aH))

    src_e = edge_index[0].astype(np.int64)
    tgt_e = edge_index[1].astype(np.int64)

    # degrees with self loops (matches PyG GCNConv gcn_norm)
    deg = np.bincount(tgt_e, minlength=N).astype(np.float64) + 1.0
    dinv = 1.0 / np.sqrt(deg)

    # full edge list: graph edges + self loops
    loops = np.arange(N, dtype=np.int64)
    srcs = np.concatenate([src_e, loops])
    tgts = np.concatenate([tgt_e, loops])
    w = np.concatenate([
        aL * dinv[src_e] * dinv[tgt_e],
        aL * dinv * dinv,
    ]).astype(np.float32) * XW_SCALE

    order = np.argsort(tgts, kind="stable")
    srcs, tgts, w = srcs[order], tgts[order], w[order]

    ngrp = N // G
    gpc = ngrp // NCORES            # groups per core
    grp = tgts // G
    counts = np.bincount(grp, minlength=ngrp)
    C = int(np.ceil(counts.max() / 128))
    QB = GPB * C
    S = MB * QB * 128

    grp_start = np.zeros(ngrp, np.int64)
    grp_start[1:] = np.cumsum(counts)[:-1]
    pos = np.arange(len(tgts)) - grp_start[grp]
    core = grp // gpc
    gic = grp % gpc                 # group index within core
    slot = gic * C * 128 + pos      # slot within the core's edge array

    # gather index (scratch-row space); padding slots point at row 0 and
    # carry zero segment weights
    scr_rows = ((srcs % 128) * KB + srcs // 128).astype(np.int16)
    gidx_all = np.zeros((NCORES, S), np.int16)
    gidx_all[core, slot] = scr_rows

    # segment matrix, partition-contiguous layout:
    # row = block*128 + slot%128, col = (group-in-block*C + chunk)*G + tgt%G
    segT_all = np.zeros((NCORES, MB * 128, QB * G), NP8M)
    blk = gic // GPB
    q = (gic % GPB) * C + pos // 128
    segT_all[core, blk * 128 + pos % 128, q * G + tgts % G] = w.astype(NP8M)

    xT = np.ascontiguousarray(x.T).astype(np.float16)
    wt2 = np.ascontiguousarray(
        np.concatenate([W_high.T, W_conv.T], axis=1)).astype(np.float16)
    bias128 = np.tile((aL * b_conv).astype(np.float32)[None, :], (128, 1))

    k16 = KB16 * 128
    in_maps = []
    for j in range(NCORES):
        lsT = np.ascontiguousarray(
            (LS_SCALE * aH * Lsym[j * M:(j + 1) * M, :]).T)
        g = gidx_all[j]
        gw = np.ascontiguousarray(g.reshape(S // 16, 16).T)  # [16, S/16]
        im = {
            "lsymT8": np.ascontiguousarray(lsT[k16:]).astype(NP8L),
            "xT": xT,
            "wt2": wt2,
            "segP": np.ascontiguousarray(
                segT_all[j].reshape(MB, 128, QB * G).transpose(1, 0, 2)
                .reshape(128, MB * QB * G)),
            "gidx": np.ascontiguousarray(np.tile(gw, (8, 1))),
            "bias128": bias128,
        }
        if k16:
            im["lsymT16"] = np.ascontiguousarray(lsT[:k16]).astype(np.float16)
        in_maps.append(im)
    return in_maps, C


_CACHE = {}


def kernel(x, edge_index, Lsym, W_high, W_conv, b_conv, aL, aH):
    in_maps, C = _prepare_host(x, edge_index, Lsym, W_high, W_conv, b_conv,
                               aL, aH)
    nc = _CACHE.get(C)
    if nc is None:
        nc = _build_program(C)
        _CACHE[C] = nc
    res = run_bass_kernel_spmd(nc, in_maps, core_ids=list(range(NCORES)))
    out = np.concatenate(
        [np.asarray(res.results[j]["out"], np.float32)
         .reshape(128, MB, D).transpose(1, 0, 2).reshape(M, D)
         for j in range(NCORES)], axis=0)
    return out
